# revision 2
# baseline (speedup 1.0000x reference)
"""MoE layer (nn_MoELayer_4681514353281) Trainium2 Bass kernel.

Reference semantics: for slot i in range(4), expert i's FFN (W1 + A1@B1 LoRA,
gelu-tanh, W2 + A2@B2 LoRA) runs densely over ALL tokens; per-token combine
weight = renormalized top-4 softmax gate weight where top_idx == i (else 0).
Only experts 0-3 are ever used, and a token contributes to expert i only when
i is among its top-4 of 16 — i.e. ~1/4 of tokens per expert. The dense
reference multiplies the other ~3/4 by zero.

This kernel exploits that sparsity: the host computes the fp32 gate (needs
~1e-6 logit precision to reproduce the reference's top-4 picks, beyond the
PE's reduced-precision multiply path; it is 0.13% of FLOPs), gathers each
expert's selected tokens into a packed array, and the device runs the dense
FFN only over packed tokens (~2100 instead of 8192 per expert).

Sharding: 8 cores = 4 experts x 2 halves of F (expert-parallel + intra-expert
F-split), so each expert's full [1024,4096]+[4096,1024] folded dense weights
stay SBUF-resident as bf16 halves. All matmuls run in bf16 (same 1 cycle/row
PE rate as fp32r, half the DMA/SBUF) with fp32 PSUM accumulation; measured
end-to-end rel err ~3e-3 vs the 2e-2 gate. Host scatters the per-expert
packed outputs back into the full [8192, 1024] result.
"""

import os
import sys

sys.path.insert(0, "/opt/trn_rl_repo")

import ml_dtypes
import numpy as np

# Problem dims (hardcoded per spec)
B, S, D, F, E, R = 2, 4096, 1024, 4096, 16, 16
TOPK = 4
N_TOK = B * S          # 8192
F2 = F // 2            # 2048 per-core F half
TOK_BLK = 512
DC = D // 128          # 8
FC = F2 // 128         # 16

BF16NP = ml_dtypes.bfloat16

_programs = {}
LAST_RESULTS = None


def _build_program(n128):
    """Program for T_pad = n128*128 packed tokens: blocks of 512 + 128-tail."""
    import concourse.tile as tile
    from concourse import bacc, mybir

    F32 = mybir.dt.float32
    BF16 = mybir.dt.bfloat16
    AF = mybir.ActivationFunctionType

    t_pad = n128 * 128
    blocks = []
    t0 = 0
    while t0 < t_pad:
        w = min(TOK_BLK, t_pad - t0)
        blocks.append((t0, w))
        t0 += w

    nc = bacc.Bacc("TRN2", target_bir_lowering=False, debug=False, num_devices=8)

    xTd = nc.dram_tensor("xT", [D, t_pad], BF16, kind="ExternalInput")
    w1d = nc.dram_tensor("w1", [D, F2], BF16, kind="ExternalInput")
    w2d = nc.dram_tensor("w2", [F2, D], BF16, kind="ExternalInput")
    wcd = nc.dram_tensor("wc", [128, n128], F32, kind="ExternalInput")
    outd = nc.dram_tensor("out", [t_pad, D], F32, kind="ExternalOutput")

    with tile.TileContext(nc) as tc:
        with (
            tc.tile_pool(name="singles", bufs=1) as singles,
            tc.tile_pool(name="xp", bufs=2) as xp,
            tc.tile_pool(name="hap", bufs=FC + 2) as hap,
            tc.tile_pool(name="outp", bufs=3) as outp,
            tc.tile_pool(name="psH", bufs=3, space="PSUM") as psH,
            tc.tile_pool(name="psEO", bufs=5, space="PSUM") as psEO,
        ):
            # ---- resident weights ----
            w1 = singles.tile([128, FC, DC, 128], BF16)   # [p, fc, dc, q]
            w2 = singles.tile([128, FC, D], BF16)         # [p, fc, d]
            w_all = singles.tile([128, n128], F32)

            xT_r = xTd.rearrange("(dc p) t -> p dc t", p=128)
            w1_r = w1d.rearrange("(dc p) (fc q) -> p fc dc q", p=128, q=128)
            w2_r = w2d.rearrange("(fc p) d -> p fc d", p=128)

            def load_block(t0, w):
                t = xp.tile([128, DC, TOK_BLK], BF16, tag="xb")
                nc.scalar.dma_start(t[:, :, :w], xT_r[:, :, t0:t0 + w])
                return t

            xb = load_block(*blocks[0])

            nc.sync.dma_start(w_all[:], wcd[:, :])
            for fc in range(FC):
                nc.sync.dma_start(w1[:, fc, :, :], w1_r[:, fc, :, :])
                nc.sync.dma_start(w2[:, fc, :], w2_r[:, fc, :])

            for blk, (t0, w) in enumerate(blocks):
                # up projection: h[fc][:, t] = gelu(x @ W1c)[f, t]
                h_all = []
                for fc in range(FC):
                    ps_h = psH.tile([128, TOK_BLK], F32)
                    for dc in range(DC):
                        nc.tensor.matmul(
                            ps_h[:, :w], w1[:, fc, dc, :], xb[:, dc, :w],
                            start=(dc == 0), stop=(dc == DC - 1),
                        )
                    h = hap.tile([128, TOK_BLK], BF16, tag="h")
                    nc.scalar.activation(h[:, :w], ps_h[:, :w], AF.Gelu_apprx_tanh)
                    h_all.append(h)

                # prefetch next block's x while the down passes run
                if blk + 1 < len(blocks):
                    xb_next = load_block(*blocks[blk + 1])
                else:
                    xb_next = None

                # down projection in two d-half passes, 128-token columns
                for dh in range(2):
                    for sub in range(w // 128):
                        eo = psEO.tile([128, 512], F32, tag="eo")
                        for fc in range(FC):
                            nc.tensor.matmul(
                                eo[:],
                                h_all[fc][:, sub * 128:(sub + 1) * 128],
                                w2[:, fc, dh * 512:(dh + 1) * 512],
                                start=(fc == 0), stop=(fc == FC - 1),
                            )
                        ob = outp.tile([128, 512], F32, tag="ob")
                        col = t0 // 128 + sub
                        nc.vector.tensor_scalar_mul(
                            ob[:], eo[:], scalar1=w_all[:, col:col + 1]
                        )
                        trow = t0 + sub * 128
                        nc.scalar.dma_start(
                            outd[trow:trow + 128, dh * 512:(dh + 1) * 512], ob[:]
                        )

                xb = xb_next

    nc.compile()
    return nc


def _get_program(n128):
    if n128 not in _programs:
        _programs[n128] = _build_program(n128)
    return _programs[n128]


def _gate_weights(x2d, Wg):
    """Reference-faithful gate (same ops as the reference, jax on CPU so the
    fp32 softmax/top-4 selection matches bit-for-bit). Returns [N_TOK, 4]
    combine weights for experts 0-3."""
    try:
        import jax
        import jax.numpy as jnp
        cpu = jax.devices("cpu")[0]
        with jax.default_device(cpu):
            xf = jnp.asarray(x2d, jnp.float32)
            wg = jnp.asarray(Wg, jnp.float32)
            weights = jax.nn.softmax(xf @ wg, axis=-1)
            top_w, top_idx = jax.lax.top_k(weights, TOPK)
            top_w = top_w / jnp.sum(top_w, axis=-1, keepdims=True)
            cols = [jnp.sum(top_w * (top_idx == i), axis=-1) for i in range(TOPK)]
            return np.asarray(jnp.stack(cols, axis=-1), np.float32)
    except Exception:
        # numpy fallback (identical math, BLAS rounding may differ ~1e-7)
        logits = x2d.astype(np.float32) @ Wg.astype(np.float32)
        m = logits.max(axis=-1, keepdims=True)
        e = np.exp((logits - m).astype(np.float32), dtype=np.float32)
        p = (e / e.sum(axis=-1, keepdims=True).astype(np.float32)).astype(np.float32)
        idx = np.argsort(-p, axis=-1, kind="stable")[:, :TOPK]
        topw = np.take_along_axis(p, idx, axis=-1)
        topw = (topw / topw.sum(axis=-1, keepdims=True)).astype(np.float32)
        w = np.zeros((x2d.shape[0], TOPK), np.float32)
        for i in range(TOPK):
            w[:, i] = (topw * (idx == i)).sum(axis=-1)
        return w


def kernel(x, Wg, W1, A1, B1, W2, A2, B2):
    global LAST_RESULTS
    from concourse.bass_utils import run_bass_kernel_spmd

    x = np.asarray(x, dtype=np.float32)
    x2d = x.reshape(N_TOK, D)
    w4 = _gate_weights(x2d, np.asarray(Wg, dtype=np.float32))

    # per-expert routed token sets (w4 > 0 iff expert i in the token's top-4)
    idx = [np.nonzero(w4[:, e] > 0)[0] for e in range(TOPK)]
    counts = [len(ix) for ix in idx]
    n128 = max(1, -(-max(counts) // 128))
    t_pad = n128 * 128

    nc = _get_program(n128)

    # per-expert packed inputs (shared by the expert's two F-half cores)
    xT_e, wc_e, w1c_e, w2c_e = [], [], [], []
    for e in range(TOPK):
        ce = counts[e]
        xpck = np.zeros((t_pad, D), dtype=BF16NP)
        xpck[:ce] = x2d[idx[e]]
        xT_e.append(np.ascontiguousarray(xpck.T))
        wc = np.zeros(t_pad, dtype=np.float32)
        wc[:ce] = w4[idx[e], e]
        # [128, n128]: column c holds packed tokens [c*128, (c+1)*128)
        wc_e.append(np.ascontiguousarray(wc.reshape(n128, 128).T))
        # fold the rank-16 LoRA into the dense weights (exact identity)
        w1c = (np.asarray(W1[e], np.float64)
               + np.asarray(A1[e], np.float64) @ np.asarray(B1[e], np.float64))
        w2c = (np.asarray(W2[e], np.float64)
               + np.asarray(A2[e], np.float64) @ np.asarray(B2[e], np.float64))
        w1c_e.append(w1c.astype(BF16NP))
        w2c_e.append(w2c.astype(BF16NP))

    in_maps = []
    for core in range(8):
        e = core % 4
        half = core // 4
        f0, f1 = half * F2, (half + 1) * F2
        in_maps.append({
            "xT": xT_e[e],
            "w1": np.ascontiguousarray(w1c_e[e][:, f0:f1]),
            "w2": np.ascontiguousarray(w2c_e[e][f0:f1, :]),
            "wc": wc_e[e],
        })

    trace = bool(os.environ.get("KERNEL_TRACE"))
    res = None
    last_exc = None
    for attempt in range(3):
        try:
            res = run_bass_kernel_spmd(
                nc, in_maps, core_ids=list(range(8)), trace=trace
            )
            break
        except Exception as exc:  # transient NRT/profiling faults — retry
            last_exc = exc
            if attempt >= 1:
                trace = False
    if res is None:
        raise last_exc
    LAST_RESULTS = res

    acc = np.zeros((N_TOK, D), dtype=np.float32)
    for e in range(TOPK):
        ce = counts[e]
        o = (np.asarray(res.results[e]["out"][:ce], np.float32)
             + np.asarray(res.results[e + 4]["out"][:ce], np.float32))
        acc[idx[e]] += o
    return acc.reshape(B, S, D)
